# revision 24
# baseline (speedup 1.0000x reference)
"""KAN layer (identity edges) Trainium2 kernel.

output[b, o] = sum_i x[b, i]  for all o  -- row-sum broadcast to (B, 1024).

Structural optimizations over a naive full-output kernel:

1. Rank-1 output: the device computes ONLY the row sums; the host
   reconstructs the broadcast during unshard (the reference's own
   jnp.broadcast_to is the same operation).  Removes the entire 256 MB
   output write from HBM.

2. fp16 ingest: the row-sum tolerates quantization easily (1024-term
   sum, fp32 accumulation; measured L2 rel err ~5e-4 vs the 2e-2 gate),
   so the host casts x to fp16 before upload, halving HBM read traffic
   to 16 MiB/core.

3. Two compute engines keep pace with the ~400-425 GB/s load stream
   (HW-probed rates):
     - DVE: within-row halving tensor_tensor adds (fp16 2x_1P packed
       mode, 2 elem/cycle) down to 128 wide + 1x reduce_sum (f32 out).
     - ACT: activation(Copy) with f32 accum_out, ~1.4 us/row, on the
       front rows of most tiles (~20 of 64 rows).
   GpSimd is deliberately NOT used: its tensor ops running concurrently
   with DVE packed ops degrade DVE ~5x (HW-traced SBUF interference).

4. Schedule: small ramp tiles (2,2,4,4) so the first data lands ~3 us
   after DMA start and compute begins immediately; tail tiles shrink to
   (3,1) and the store is split so only the last 4 sums trail the final
   reduce (~4 us post-load tail).
"""

import numpy as np

import concourse.tile as tile
from concourse import bacc, mybir
from concourse.bass_utils import run_bass_kernel_spmd

N_CORES = 8
BATCH = 65536
FEAT = 1024
ROWS = BATCH // N_CORES        # 8192 rows per core
P = 128                        # SBUF partitions
ROWS_PER_PART = ROWS // P      # 64 consecutive rows owned by each partition

R_SCHED = (2, 2, 4, 4, 8, 8, 8, 8, 8, 4, 4, 2, 1, 1)
RING_B = ()                            # ALL loads on the sync ring.  Any load
                                       # issued from the scalar engine -- even a
                                       # ramp tile ahead of all ACT compute, even
                                       # under tc.high_priority() -- measurably
                                       # stalls the load stream (HW-traced three
                                       # times: +5-11us).  One HWDGE queue
                                       # sustains ~400-425 GB/s on its own.
ACT_ROWS = (0, 2, 0, 2, 3, 3, 2, 2, 3, 0, 2, 0, 1, 1)  # rows from tile front on ACT
GPS_TILES = ()                         # a SWDGE (nc.gpsimd) second DMA queue was
                                       # HW-tested: no DVE interference, but
                                       # aggregate DMA dropped to ~373 GB/s (the
                                       # 16 SDMA engines are the shared ceiling,
                                       # ~430 GB/s; extra queues only add
                                       # round-robin overhead).  GpSimd compute
                                       # is also banned: concurrent GpSimd TT
                                       # degrades DVE packed ops 5x (HW-traced).
IN_BUFS = 10
F16 = mybir.dt.float16
F32 = mybir.dt.float32

_nc_cache = []


def _dve_tree(nc, t, r0, r1, h1, h2, h3, s_all, lo):
    """Row sums of t[:, r0:r1, :1024] -> s_all[:, lo:lo+(r1-r0)] on DVE."""
    n = r1 - r0
    if n == 1:
        nc.vector.reduce_sum(
            out=s_all[:, lo : lo + 1], in_=t[:, r0, :], axis=mybir.AxisListType.X
        )
        return
    nc.vector.tensor_add(
        out=h1[:, 0:n, :], in0=t[:, r0:r1, 0:512], in1=t[:, r0:r1, 512:1024]
    )
    nc.vector.tensor_add(
        out=h2[:, 0:n, :], in0=h1[:, 0:n, 0:256], in1=h1[:, 0:n, 256:512]
    )
    nc.vector.tensor_add(
        out=h3[:, 0:n, :], in0=h2[:, 0:n, 0:128], in1=h2[:, 0:n, 128:256]
    )
    nc.vector.reduce_sum(
        out=s_all[:, lo : lo + n], in_=h3[:, 0:n, :], axis=mybir.AxisListType.X
    )


def _build():
    assert sum(R_SCHED) == ROWS_PER_PART
    nc = bacc.Bacc()
    x = nc.declare_dram_parameter("x", [ROWS, FEAT], F16, isOutput=False)
    y = nc.declare_dram_parameter("y", [P, ROWS_PER_PART], F32, isOutput=True)
    xv = x[:, :].rearrange("(p n) d -> p n d", p=P)

    max_r = max(R_SCHED)
    max_act = 4

    with tile.TileContext(nc) as tc:
        with (
            tc.tile_pool(name="inp", bufs=IN_BUFS) as inp,
            tc.tile_pool(name="sums", bufs=1) as sums_pool,
            tc.tile_pool(name="tree", bufs=1) as tree_pool,
            tc.tile_pool(name="scr", bufs=1) as scr_pool,
        ):
            s_all = sums_pool.tile([P, ROWS_PER_PART], F32, tag="s")
            h1 = tree_pool.tile([P, max_r, 512], F16, tag="h1")
            h2 = tree_pool.tile([P, max_r, 256], F16, tag="h2")
            h3 = tree_pool.tile([P, max_r, 128], F16, tag="h3")
            scr = scr_pool.tile([P, max_act, FEAT], F16, tag="scr")

            # --- all loads first (ring issues precede any compute in
            # each sequencer's FIFO)
            tiles, rows_of = [], []
            row = 0
            for i, r in enumerate(R_SCHED):
                t = inp.tile([P, r, FEAT], F16, tag="in")
                eng = nc.scalar if i in RING_B else nc.sync
                eng.dma_start(out=t[:, :, :], in_=xv[:, row : row + r, :])
                tiles.append(t)
                rows_of.append(row)
                row += r

            # --- ACT rows (front rows of its tiles, ascending)
            for i, r in enumerate(R_SCHED):
                for j in range(ACT_ROWS[i]):
                    nc.scalar.activation(
                        out=scr[:, j % max_act, :],
                        in_=tiles[i][:, j, :],
                        func=mybir.ActivationFunctionType.Copy,
                        accum_out=s_all[:, rows_of[i] + j : rows_of[i] + j + 1],
                    )

            # --- DVE trees (ascending; tail tiles land last anyway)
            for i, r in enumerate(R_SCHED):
                a = ACT_ROWS[i]
                if r - a > 0:
                    _dve_tree(nc, tiles[i], a, r, h1, h2, h3, s_all, rows_of[i] + a)

            # split store: rows 0..59 (tiles 0-9) go out as soon as their
            # reduces land; only the 4 tail-tile sums trail the last reduce
            nc.sync.dma_start(out=y[:, 0:60], in_=s_all[:, 0:60])
            nc.sync.dma_start(out=y[:, 60:64], in_=s_all[:, 60:64])
    nc.finalize()
    return nc


def _get_nc():
    if not _nc_cache:
        _nc_cache.append(_build())
    return _nc_cache[0]


def kernel(x: np.ndarray) -> np.ndarray:
    nc = _get_nc()
    xh = np.ascontiguousarray(np.asarray(x)).astype(np.float16)
    shards = np.split(xh, N_CORES, axis=0)
    in_maps = [{"x": s} for s in shards]
    res = run_bass_kernel_spmd(nc, in_maps, list(range(N_CORES)))
    sums = np.concatenate(
        [res.results[i]["y"].reshape(ROWS) for i in range(N_CORES)], axis=0
    )
    return np.ascontiguousarray(np.broadcast_to(sums[:, None], (BATCH, FEAT)))
